# revision 2
# baseline (speedup 1.0000x reference)
"""MixedArityTreeLSTM Trainium2 kernel.

Level-synchronous bottom-up Tree-LSTM over B=256 heap-indexed perfect binary
trees (511 nodes, depth 8), E=H=128. Pure data-parallel over 8 NeuronCores
(32 trees per core); all weights replicated.

v2: no device-side gather. The host packs all token-dependent data as dense
feature-major bf16 streams (pure data movement — all arithmetic either stays
on device or is a vocab-indexed weight-only table transform):
  - hleaf:  rows of the weight-only table tanh(emb @ W3 + bW3), giving the
            leaf level h directly.
  - xT:     emb rows for the internal-node tokens, [E, nodes] per level.
Device per internal level (feature-major [H(part), nodes(free)]):
    pre_g = W_g^T x + Ubt_g^T (m*h_l) + Ubb_g^T (m*h_r) + Uun_g^T ((1-m)*h_l)
            + m * (b_bin_g - b_un_g)   [K=1 outer-product matmul]
            + (bW_g + b_un_g)          [ACT bias]
Matmul operands bf16; PSUM/c fp32; gate activations bf16.
"""

import numpy as np
import ml_dtypes

B, D = 256, 8
V, E, H = 32000, 128, 128
N_NODES = 2 ** (D + 1) - 1  # 511
NCORES = 8
BL = B // NCORES  # 32 trees per core

# internal levels in storage order L7..L0; LVL_OFF = column offset in concat
LVL_N = {l: BL * (2**l) for l in range(D + 1)}
INT_LEVELS = list(range(D - 1, -1, -1))  # 7..0
LVL_OFF = {}
_off = 0
for _l in INT_LEVELS:
    LVL_OFF[_l] = _off
    _off += LVL_N[_l]
XCOLS = _off  # 8160
LEAF_COLS = LVL_N[D]  # 8192

# chunks per level (chunk = up to 512 cols)
CPL = {l: max(1, LVL_N[l] // 512) for l in range(D + 1)}

# post-order dependency wave over the chunk tree: children before parent
ORDER = []


def _post(l, j):
    if l < D:
        if CPL[l + 1] == 2 * CPL[l]:
            _post(l + 1, 2 * j)
            _post(l + 1, 2 * j + 1)
        else:
            assert CPL[l + 1] == CPL[l] == 1
            _post(l + 1, 0)
    ORDER.append((l, j))


_post(0, 0)

# internal-level compute chunks in wave order: (cid, lvl, c0, N)
CHUNKS = []
for lvl, j in ORDER:
    if lvl == D:
        continue
    N = min(512, LVL_N[lvl] - j * 512)
    CHUNKS.append((len(CHUNKS), lvl, j * 512, N))

BF16 = ml_dtypes.bfloat16

_CACHE = {}


def _build_nc():
    """Build the (SPMD, per-core) Bass/Tile kernel. Cached per process."""
    if "nc" in _CACHE:
        return _CACHE["nc"]

    from contextlib import ExitStack

    import concourse.mybir as mybir
    import concourse.tile as tile
    from concourse import bacc

    dt = mybir.dt
    AF = mybir.ActivationFunctionType

    nc = bacc.Bacc()

    hleaf_d = nc.dram_tensor("hleaf", [128, LEAF_COLS], dt.bfloat16, kind="ExternalInput")
    xall_d = nc.dram_tensor("xall", [128, XCOLS], dt.bfloat16, kind="ExternalInput")
    mbc_d = nc.dram_tensor("mbcast", [128, XCOLS], dt.bfloat16, kind="ExternalInput")
    maskb_d = nc.dram_tensor("maskb", [1, XCOLS], dt.bfloat16, kind="ExternalInput")
    w_d = nc.dram_tensor("w_bf", [4, E, H], dt.bfloat16, kind="ExternalInput")
    ubt_d = nc.dram_tensor("ubt_bf", [5, H, H], dt.bfloat16, kind="ExternalInput")
    ubb_d = nc.dram_tensor("ubb_bf", [5, H, H], dt.bfloat16, kind="ExternalInput")
    uun_d = nc.dram_tensor("uun_bf", [4, H, H], dt.bfloat16, kind="ExternalInput")
    # bias rows: 0=unused 1=bc_i 2=bc_fL 3=b_fR 4=bc_o 5=bc_u
    bias_d = nc.dram_tensor("biases", [6, H], dt.float32, kind="ExternalInput")
    # delta rows: 0=d_i 1=d_fL 2=d_o 3=d_u 4=+40 (f_r unary kill)
    delt_d = nc.dram_tensor("deltas", [5, H], dt.bfloat16, kind="ExternalInput")

    h_out_d = nc.dram_tensor("h_out", [H, BL], dt.float32, kind="ExternalOutput")
    c_out_d = nc.dram_tensor("c_out", [H, BL], dt.float32, kind="ExternalOutput")

    with tile.TileContext(nc) as tc, ExitStack() as ctx:
        consts = ctx.enter_context(tc.tile_pool(name="consts", bufs=1))

        # small consts on the Act HWDGE queue (scalar engine is idle at t=0)
        w_sb = consts.tile([E, 4, H], dt.bfloat16)
        nc.scalar.dma_start(out=w_sb, in_=w_d[:, :, :].rearrange("g e h -> e g h"))
        ubt_sb = consts.tile([H, 5, H], dt.bfloat16)
        nc.scalar.dma_start(out=ubt_sb, in_=ubt_d[:, :, :].rearrange("g k h -> k g h"))
        ubb_sb = consts.tile([H, 5, H], dt.bfloat16)
        nc.scalar.dma_start(out=ubb_sb, in_=ubb_d[:, :, :].rearrange("g k h -> k g h"))
        uun_sb = consts.tile([H, 4, H], dt.bfloat16)
        nc.scalar.dma_start(out=uun_sb, in_=uun_d[:, :, :].rearrange("g k h -> k g h"))
        bias_sb = consts.tile([H, 6], dt.float32)
        nc.scalar.dma_start(out=bias_sb, in_=bias_d[:, :].rearrange("n h -> h n"))
        delt_sb = consts.tile([1, 5, H], dt.bfloat16)
        nc.scalar.dma_start(
            out=delt_sb, in_=delt_d[:, :].rearrange("(o g) h -> o g h", o=1)
        )
        maskb_sb = consts.tile([1, XCOLS], dt.bfloat16)
        nc.scalar.dma_start(out=maskb_sb, in_=maskb_d[:, :])

        lev = ctx.enter_context(tc.tile_pool(name="lev", bufs=1))

        # broadcast mask, 4 pieces on the Pool (gpsimd) queue
        mbc_sb = lev.tile([128, XCOLS], dt.bfloat16, name="mbc", tag="mbc")
        for a, b in ((0, 2048), (2048, 4096), (4096, 6144), (6144, XCOLS)):
            nc.gpsimd.dma_start(out=mbc_sb[:, a:b], in_=mbc_d[:, a:b])

        # x for internal levels, one tile; level slices DMA'd in wave order
        xt = lev.tile([128, XCOLS], dt.bfloat16, name="xT", tag="xT")

        # --- working pools ---
        psum = ctx.enter_context(tc.tile_pool(name="psum", bufs=8, space="PSUM"))
        work = ctx.enter_context(tc.tile_pool(name="work", bufs=2))

        h_t = {}
        c_t = {}
        h_t[D] = lev.tile([H, LEAF_COLS], dt.bfloat16, name="h_leaf", tag="h_leaf")

        x_issued = set()

        for lvl, j in ORDER:
            if lvl == D:
                # leaf chunk: stream precomputed h
                c0 = j * 512
                nc.sync.dma_start(
                    out=h_t[D][:, c0 : c0 + 512], in_=hleaf_d[:, c0 : c0 + 512]
                )
                continue

            if lvl not in x_issued:
                # stream this level's x (split L7 into two 2048-col pieces)
                x_issued.add(lvl)
                lo = LVL_OFF[lvl]
                n = LVL_N[lvl]
                if n > 2048:
                    for a in range(lo, lo + n, 2048):
                        nc.sync.dma_start(
                            out=xt[:, a : a + 2048], in_=xall_d[:, a : a + 2048]
                        )
                else:
                    nc.sync.dma_start(out=xt[:, lo : lo + n], in_=xall_d[:, lo : lo + n])

            c0 = j * 512
            N = min(512, LVL_N[lvl] - c0)
            first_chunk = c0 == 0
            if first_chunk:
                n = LVL_N[lvl]
                hdt = dt.float32 if lvl == 0 else dt.bfloat16
                h_t[lvl] = lev.tile([H, n], hdt, name=f"h_l{lvl}", tag=f"h_l{lvl}")
                c_t[lvl] = lev.tile([H, n], dt.float32, name=f"c_l{lvl}", tag=f"c_l{lvl}")

            hch = h_t[lvl + 1]
            pairs = hch[:, 2 * c0 : 2 * c0 + 2 * N].rearrange(
                "p (n two) -> p n two", two=2
            )
            h_e, h_o = pairs[:, :, 0], pairs[:, :, 1]

            moff = LVL_OFF[lvl] + c0
            mb = mbc_sb[:, moff : moff + N]

            heb = work.tile([128, N], dt.bfloat16, tag="heb", name="heb")
            nc.vector.tensor_mul(heb, h_e, mb)
            hob = work.tile([128, N], dt.bfloat16, tag="hob", name="hob")
            nc.vector.tensor_mul(hob, h_o, mb)
            heu = work.tile([128, N], dt.bfloat16, tag="heu", name="heu")
            nc.vector.tensor_sub(heu, h_e, heb)

            xs = xt[:, moff : moff + N]
            mrow = maskb_sb[:, moff : moff + N]
            top = lvl == D - 1  # children are leaves: c=0, skip f gates

            # gate -> (W idx, Ubin idx, Uun idx or None, delta idx or None)
            if top:
                gates = [("i", 0, 0, 0, 0), ("o", 2, 3, 2, 2), ("u", 3, 4, 3, 3)]
            else:
                gates = [
                    ("i", 0, 0, 0, 0),
                    ("fl", 1, 1, 1, 1),
                    ("fr", 1, 2, None, 4),
                    ("o", 2, 3, 2, 2),
                    ("u", 3, 4, 3, 3),
                ]

            pts = {}
            for gname, wi, ubi, uui, di in gates:
                ps = psum.tile([H, N], dt.float32, tag="pg", name=f"ps_{gname}")
                nc.tensor.matmul(ps, w_sb[:, wi, :], xs, start=True, stop=False)
                nc.tensor.matmul(ps, ubt_sb[:, ubi, :], heb, start=False, stop=False)
                nc.tensor.matmul(ps, ubb_sb[:, ubi, :], hob, start=False, stop=False)
                if uui is not None:
                    nc.tensor.matmul(ps, uun_sb[:, uui, :], heu, start=False, stop=False)
                nc.tensor.matmul(ps, delt_sb[:, di, :], mrow, start=False, stop=True)
                pts[gname] = ps

            gi = work.tile([128, N], dt.bfloat16, tag="gi", name="gi")
            nc.scalar.activation(gi, pts["i"], AF.Sigmoid, bias=bias_sb[:, 1:2])
            go = work.tile([128, N], dt.bfloat16, tag="go", name="go")
            nc.scalar.activation(go, pts["o"], AF.Sigmoid, bias=bias_sb[:, 4:5])
            gu = work.tile([128, N], dt.bfloat16, tag="gu", name="gu")
            nc.scalar.activation(gu, pts["u"], AF.Tanh, bias=bias_sb[:, 5:6])

            cs = c_t[lvl][:, c0 : c0 + N]
            if top:
                nc.vector.tensor_mul(cs, gi, gu)
            else:
                gfl = work.tile([128, N], dt.bfloat16, tag="gfl", name="gfl")
                nc.scalar.activation(gfl, pts["fl"], AF.Sigmoid, bias=bias_sb[:, 2:3])
                gfr = work.tile([128, N], dt.bfloat16, tag="gfr", name="gfr")
                nc.scalar.activation(gfr, pts["fr"], AF.Sigmoid, bias=bias_sb[:, 3:4])
                cch = c_t[lvl + 1]
                cpairs = cch[:, 2 * c0 : 2 * c0 + 2 * N].rearrange(
                    "p (n two) -> p n two", two=2
                )
                c_e, c_o = cpairs[:, :, 0], cpairs[:, :, 1]

                t1 = work.tile([128, N], dt.float32, tag="t1", name="t1")
                nc.vector.tensor_mul(t1, gi, gu)
                t2 = work.tile([128, N], dt.float32, tag="t2", name="t2")
                nc.vector.tensor_mul(t2, gfl, c_e)
                nc.vector.tensor_add(cs, t1, t2)
                t3 = work.tile([128, N], dt.float32, tag="t3", name="t3")
                nc.vector.tensor_mul(t3, gfr, c_o)
                nc.vector.tensor_add(cs, cs, t3)

            tch = work.tile([128, N], dt.float32, tag="tch", name="tch")
            nc.scalar.activation(tch, cs, AF.Tanh)
            nc.vector.tensor_mul(h_t[lvl][:, c0 : c0 + N], go, tch)

        nc.sync.dma_start(out=h_out_d[:, :], in_=h_t[0][:, :BL])
        nc.sync.dma_start(out=c_out_d[:, :], in_=c_t[0][:, :BL])

    nc.finalize()
    _CACHE["nc"] = nc
    return nc


def prep_core_inputs(tokens_c, arity_c, shared):
    """Per-core input map. tokens_c [BL,511], arity_c [BL,255].

    Host work is pure data movement: gather rows of precomputed vocab tables
    and pack masks; no model arithmetic happens here.
    """
    tokens_c = np.asarray(tokens_c)
    arity_c = np.asarray(arity_c, np.int32)
    emb_bf = shared["_emb_bf"]
    hleaf_tab = shared["_hleaf_tab"]

    # leaf h rows, feature-major
    leaf_toks = tokens_c[:, 2**D - 1 :].reshape(-1)  # [8192]
    hleaf = np.ascontiguousarray(hleaf_tab[leaf_toks].T)  # [128, 8192]

    # internal x rows, feature-major, levels L7..L0 concatenated
    xcols = []
    mcols = []
    for lvl in INT_LEVELS:
        off, cnt = 2**lvl - 1, 2**lvl
        toks = tokens_c[:, off : off + cnt].reshape(-1)
        xcols.append(emb_bf[toks].T)
        mcols.append((arity_c[:, off : off + cnt].reshape(-1) == 1).astype(BF16))
    xall = np.ascontiguousarray(np.concatenate(xcols, axis=1))  # [128, 8160]
    maskb = np.concatenate(mcols)[None, :]  # [1, 8160]

    out = {k: v for k, v in shared.items() if not k.startswith("_")}
    out.update(
        hleaf=hleaf,
        xall=xall,
        maskb=maskb,
        mbcast=np.broadcast_to(maskb, (128, XCOLS)).copy(),
    )
    return out


def prep_shared_inputs(emb, W, bW, Ubin, bUbin, Uun, bUun):
    emb = np.asarray(emb, np.float32)
    W = np.asarray(W, np.float32)
    bW = np.asarray(bW, np.float32)
    Ubin = np.asarray(Ubin, np.float32)
    bUbin = np.asarray(bUbin, np.float32)
    Uun = np.asarray(Uun, np.float32)
    bUun = np.asarray(bUun, np.float32)

    biases = np.stack(
        [
            bW[3],                # unused (leaf folded into table)
            bW[0] + bUun[0],      # i common
            bW[1] + bUun[1],      # fL common
            bW[1] + bUbin[2] - 40.0,  # fR (binary-only; -40 kills unary)
            bW[2] + bUun[2],      # o common
            bW[3] + bUun[3],      # u common
        ]
    ).astype(np.float32)
    deltas = np.stack(
        [
            bUbin[0] - bUun[0],
            bUbin[1] - bUun[1],
            bUbin[3] - bUun[2],
            bUbin[4] - bUun[3],
            np.full(H, 40.0, np.float32),
        ]
    ).astype(BF16)

    emb_bf = emb.astype(BF16)
    # weight-only vocab table: leaf h = tanh(emb @ W3 + bW3)
    hleaf_tab = np.tanh(emb @ W[3] + bW[3]).astype(BF16)  # [V, 128]

    return dict(
        _emb_bf=emb_bf,
        _hleaf_tab=hleaf_tab,
        w_bf=W.astype(BF16),
        ubt_bf=Ubin[:, :H, :].astype(BF16),
        ubb_bf=Ubin[:, H:, :].astype(BF16),
        uun_bf=Uun.astype(BF16),
        biases=biases,
        deltas=deltas,
    )


def kernel(tokens, arity, emb, W, bW, Ubin, bUbin, Uun, bUun):
    from concourse.bass_utils import run_bass_kernel_spmd

    tokens = np.asarray(tokens)
    arity = np.asarray(arity)

    shared = prep_shared_inputs(emb, W, bW, Ubin, bUbin, Uun, bUun)
    in_maps = [
        prep_core_inputs(
            tokens[k * BL : (k + 1) * BL], arity[k * BL : (k + 1) * BL], shared
        )
        for k in range(NCORES)
    ]

    nc = _build_nc()
    res = run_bass_kernel_spmd(nc, in_maps, core_ids=list(range(NCORES)))
    results = res.results

    h = np.concatenate([r["h_out"].T for r in results], axis=0)
    c = np.concatenate([r["c_out"].T for r in results], axis=0)
    return h.astype(np.float32), c.astype(np.float32)


# revision 6
# speedup vs baseline: 1.2140x; 1.2140x over previous
"""MixedArityTreeLSTM Trainium2 kernel.

Level-synchronous bottom-up Tree-LSTM over B=256 heap-indexed perfect binary
trees (511 nodes, depth 8), E=H=128. Pure data-parallel over 8 NeuronCores
(32 trees per core); all weights replicated.

v3: no device-side gather; host packs token-dependent data as dense
feature-major bf16 streams (pure data movement; arithmetic stays on device or
is a vocab-indexed weight-only table transform):
  - hleaf: rows of the weight-only table tanh(emb @ W3 + bW3)  -> leaf h.
  - xT:    emb rows for internal-node tokens, [E, nodes] per level.
Device per internal level (feature-major [H(part), nodes(free)]):
    pre_g = W_g^T x + Ubt_g^T (m*h_l) + Ubb_g^T (m*h_r) + Uun_g^T ((1-m)*h_l)
            + [b_g; d_g]^T [ones; m]      [K=2 matmul: bias + arity delta]
Gate pairs share PSUM tiles so one activation covers two gates. The unary
f_r kill uses cob = m * c_r (so f_r's junk output is zeroed by the mask)
instead of a +-40 bias hack. All elementwise work is bf16-in/bf16-out in
SBUF to hit the DVE fast path; c is bf16 except the root level.
Emission is software-pipelined (masks / body / chain phases) and levels <=4
split into two independent half-batches (trees 0-15 / 16-31) that ping-pong.
"""

import numpy as np
import ml_dtypes

B, D = 256, 8
V, E, H = 32000, 128, 128
NCORES = 8
BL = B // NCORES  # 32 trees per core

LVL_N = {l: BL * (2**l) for l in range(D + 1)}
INT_LEVELS = list(range(D - 1, -1, -1))  # 7..0
LVL_OFF = {}
_off = 0
for _l in INT_LEVELS:
    LVL_OFF[_l] = _off
    _off += LVL_N[_l]
XCOLS = _off  # 8160
LEAF_COLS = LVL_N[D]  # 8192

# chunks per level: big levels 512-wide, levels <=4 split in two halves
CPL = {8: 16, 7: 8, 6: 4, 5: 2, 4: 2, 3: 2, 2: 2, 1: 2, 0: 2}
CW = {l: LVL_N[l] // CPL[l] for l in range(D + 1)}  # chunk width

# big-phase wave order (post-order over levels >=5), then tail ping-pong
BIG_SEQ = [
    (7, 0), (7, 1), (6, 0), (7, 2), (7, 3), (6, 1), (5, 0),
    (7, 4), (7, 5), (6, 2), (7, 6), (7, 7), (6, 3), (5, 1),
]
TAIL_SEQ = [(l, j) for l in (4, 3, 2, 1, 0) for j in (0, 1)]
SEQ = BIG_SEQ + TAIL_SEQ


def _children(lvl, j):
    """Child chunks (lvl+1, ...) whose h/c this chunk consumes."""
    if lvl == D - 1:
        return []  # children are leaves (DMA'd, no chain)
    lo = 2 * j * CW[lvl]
    hi = lo + 2 * CW[lvl]
    out = []
    for jj in range(CPL[lvl + 1]):
        a, b = jj * CW[lvl + 1], (jj + 1) * CW[lvl + 1]
        if a < hi and b > lo:
            out.append((lvl + 1, jj))
    return out


BF16 = ml_dtypes.bfloat16

_CACHE = {}


def _build_nc():
    if "nc" in _CACHE:
        return _CACHE["nc"]

    from contextlib import ExitStack

    import concourse.mybir as mybir
    import concourse.tile as tile
    from concourse import bacc

    dt = mybir.dt
    AF = mybir.ActivationFunctionType

    nc = bacc.Bacc()

    hleaf_d = nc.dram_tensor("hleaf", [128, LEAF_COLS], dt.bfloat16, kind="ExternalInput")
    xall_d = nc.dram_tensor("xall", [128, XCOLS], dt.bfloat16, kind="ExternalInput")
    mbc_d = nc.dram_tensor("mbcast", [128, XCOLS], dt.bfloat16, kind="ExternalInput")
    mk2_d = nc.dram_tensor("mk2", [2, XCOLS], dt.bfloat16, kind="ExternalInput")
    w_d = nc.dram_tensor("w_bf", [4, E, H], dt.bfloat16, kind="ExternalInput")
    ubt_d = nc.dram_tensor("ubt_bf", [5, H, H], dt.bfloat16, kind="ExternalInput")
    ubb_d = nc.dram_tensor("ubb_bf", [5, H, H], dt.bfloat16, kind="ExternalInput")
    uun_d = nc.dram_tensor("uun_bf", [4, H, H], dt.bfloat16, kind="ExternalInput")
    # bd rows: [2, gate, H]: row0 = common bias b_g, row1 = arity delta d_g
    bd_d = nc.dram_tensor("bd_bf", [2, 5, H], dt.bfloat16, kind="ExternalInput")

    h_out_d = nc.dram_tensor("h_out", [H, BL], dt.float32, kind="ExternalOutput")
    c_out_d = nc.dram_tensor("c_out", [H, BL], dt.float32, kind="ExternalOutput")

    with tile.TileContext(nc) as tc, ExitStack() as ctx:
        consts = ctx.enter_context(tc.tile_pool(name="consts", bufs=1))

        w_sb = consts.tile([E, 4, H], dt.bfloat16)
        nc.sync.dma_start(out=w_sb, in_=w_d[:, :, :].rearrange("g e h -> e g h"))
        ubt_sb = consts.tile([H, 5, H], dt.bfloat16)
        nc.sync.dma_start(out=ubt_sb, in_=ubt_d[:, :, :].rearrange("g k h -> k g h"))
        ubb_sb = consts.tile([H, 5, H], dt.bfloat16)
        nc.sync.dma_start(out=ubb_sb, in_=ubb_d[:, :, :].rearrange("g k h -> k g h"))
        uun_sb = consts.tile([H, 4, H], dt.bfloat16)
        nc.sync.dma_start(out=uun_sb, in_=uun_d[:, :, :].rearrange("g k h -> k g h"))
        bd_sb = consts.tile([2, 5, H], dt.bfloat16)
        nc.sync.dma_start(out=bd_sb, in_=bd_d[:, :, :])

        lev = ctx.enter_context(tc.tile_pool(name="lev", bufs=1))

        mbc_sb = lev.tile([128, XCOLS], dt.bfloat16, name="mbc", tag="mbc")
        mk2_sb = lev.tile([2, XCOLS], dt.bfloat16, name="mk2", tag="mk2")
        nc.gpsimd.dma_start(out=mk2_sb[:, 0:4096], in_=mk2_d[:, 0:4096])
        nc.gpsimd.dma_start(out=mk2_sb[:, 4096:XCOLS], in_=mk2_d[:, 4096:XCOLS])
        for a, b in ((0, 2048), (2048, 4096), (4096, 6144), (6144, XCOLS)):
            nc.gpsimd.dma_start(out=mbc_sb[:, a:b], in_=mbc_d[:, a:b])

        xt = lev.tile([128, XCOLS], dt.bfloat16, name="xT", tag="xT")

        h_t = {}
        c_t = {}
        h_t[D] = lev.tile([H, LEAF_COLS], dt.bfloat16, name="h_leaf", tag="h_leaf")
        for lvl in INT_LEVELS:
            n = LVL_N[lvl]
            hdt = dt.float32 if lvl == 0 else dt.bfloat16
            h_t[lvl] = lev.tile([H, n], hdt, name=f"h_l{lvl}", tag=f"h_l{lvl}")
            c_t[lvl] = lev.tile([H, n], hdt, name=f"c_l{lvl}", tag=f"c_l{lvl}")

        # input streams on the SP queue: leaves + x, roughly in need order
        for j in range(4):
            nc.sync.dma_start(
                out=h_t[D][:, j * 512 : j * 512 + 512],
                in_=hleaf_d[:, j * 512 : j * 512 + 512],
            )
        nc.sync.dma_start(out=xt[:, 0:2048], in_=xall_d[:, 0:2048])
        for j in range(4, 8):
            nc.sync.dma_start(
                out=h_t[D][:, j * 512 : j * 512 + 512],
                in_=hleaf_d[:, j * 512 : j * 512 + 512],
            )
        nc.sync.dma_start(out=xt[:, 2048:4096], in_=xall_d[:, 2048:4096])
        for j in range(8, 16):
            nc.sync.dma_start(
                out=h_t[D][:, j * 512 : j * 512 + 512],
                in_=hleaf_d[:, j * 512 : j * 512 + 512],
            )
        nc.sync.dma_start(out=xt[:, 4096:6144], in_=xall_d[:, 4096:6144])
        nc.sync.dma_start(out=xt[:, 6144:XCOLS], in_=xall_d[:, 6144:XCOLS])

        # PSUM: big tags (5 banks) + tail-odd parity tags (3 banks) = 8 banks
        psum = ctx.enter_context(tc.tile_pool(name="psum", bufs=1, space="PSUM"))
        work = ctx.enter_context(tc.tile_pool(name="work", bufs=4))

        state = {}  # per-chunk tiles shared between phases

        def phase_masks(lvl, j):
            N = CW[lvl]
            c0 = j * N
            moff = LVL_OFF[lvl] + c0
            top = lvl == D - 1
            hch = h_t[lvl + 1]
            pairs = hch[:, 2 * c0 : 2 * c0 + 2 * N].rearrange(
                "p (n two) -> p n two", two=2
            )
            h_e, h_o = pairs[:, :, 0], pairs[:, :, 1]
            mb = mbc_sb[:, moff : moff + N]

            heb = work.tile([128, N], dt.bfloat16, tag="heb", name="heb")
            nc.vector.tensor_mul(heb, h_e, mb)
            hob = work.tile([128, N], dt.bfloat16, tag="hob", name="hob")
            nc.vector.tensor_mul(hob, h_o, mb)
            heu = work.tile([128, N], dt.bfloat16, tag="heu", name="heu")
            nc.vector.tensor_sub(heu, h_e, heb)
            st = {"heb": heb, "hob": hob, "heu": heu}
            if not top:
                cch = c_t[lvl + 1]
                cpairs = cch[:, 2 * c0 : 2 * c0 + 2 * N].rearrange(
                    "p (n two) -> p n two", two=2
                )
                st["c_e"] = cpairs[:, :, 0]
                cob = work.tile([128, N], dt.bfloat16, tag="cob", name="cob")
                nc.vector.tensor_mul(cob, cpairs[:, :, 1], mb)
                st["cob"] = cob
            state[(lvl, j)] = st

        def phase_body(lvl, j):
            N = CW[lvl]
            c0 = j * N
            moff = LVL_OFF[lvl] + c0
            top = lvl == D - 1
            st = state[(lvl, j)]
            heb, hob, heu = st["heb"], st["hob"], st["heu"]
            xs = xt[:, moff : moff + N]
            mk = mk2_sb[:, moff : moff + N]
            # tail-odd chunks use the parity tag set (1 bank each)
            odd = lvl <= 4 and j == 1
            tagA = "tA1" if odd else "bgA"
            tagB = "tB1" if odd else "bgB"
            tagU = "tU1" if odd else "bgu"

            def accum(ps, wi, ubi, uui, gi_, label):
                nc.tensor.matmul(ps, w_sb[:, wi, :], xs, start=True, stop=False)
                nc.tensor.matmul(ps, ubt_sb[:, ubi, :], heb, start=False, stop=False)
                nc.tensor.matmul(ps, ubb_sb[:, ubi, :], hob, start=False, stop=False)
                if uui is not None:
                    nc.tensor.matmul(ps, uun_sb[:, uui, :], heu, start=False, stop=False)
                nc.tensor.matmul(ps, bd_sb[:, gi_, :], mk, start=False, stop=True)

            pA = psum.tile([H, 2 * N], dt.float32, tag=tagA, name=f"pA{lvl}_{j}")
            if top:
                # pair (i, o); no f gates (children have c = 0)
                accum(pA[:, 0:N], 0, 0, 0, 0, "i")
                accum(pA[:, N : 2 * N], 2, 3, 2, 3, "o")
                gAB = work.tile([128, 2 * N], dt.bfloat16, tag="gAB", name="gAB")
                nc.scalar.activation(gAB, pA, AF.Sigmoid)
                st["gi"], st["go"] = gAB[:, 0:N], gAB[:, N : 2 * N]
            else:
                # pair (i, fl) then pair (fr, o)
                accum(pA[:, 0:N], 0, 0, 0, 0, "i")
                accum(pA[:, N : 2 * N], 1, 1, 1, 1, "fl")
                gAB = work.tile([128, 2 * N], dt.bfloat16, tag="gAB", name="gAB")
                nc.scalar.activation(gAB, pA, AF.Sigmoid)
                st["gi"], st["gfl"] = gAB[:, 0:N], gAB[:, N : 2 * N]

                pB = psum.tile([H, 2 * N], dt.float32, tag=tagB, name=f"pB{lvl}_{j}")
                accum(pB[:, 0:N], 1, 2, None, 2, "fr")
                accum(pB[:, N : 2 * N], 2, 3, 2, 3, "o")
                gFO = work.tile([128, 2 * N], dt.bfloat16, tag="gFO", name="gFO")
                nc.scalar.activation(gFO, pB, AF.Sigmoid)
                st["gfr"], st["go"] = gFO[:, 0:N], gFO[:, N : 2 * N]

            pU = psum.tile([H, N], dt.float32, tag=tagU, name=f"pU{lvl}_{j}")
            accum(pU, 3, 4, 3, 4, "u")
            gu = work.tile([128, N], dt.bfloat16, tag="gu", name="gu")
            nc.scalar.activation(gu, pU, AF.Tanh)
            st["gu"] = gu

        def phase_chain(lvl, j):
            N = CW[lvl]
            c0 = j * N
            top = lvl == D - 1
            st = state.pop((lvl, j))
            cs = c_t[lvl][:, c0 : c0 + N]
            wdt = dt.float32 if lvl == 0 else dt.bfloat16
            if top:
                nc.vector.tensor_mul(cs, st["gi"], st["gu"])
            else:
                t1 = work.tile([128, N], wdt, tag="t1", name="t1")
                nc.vector.tensor_mul(t1, st["gi"], st["gu"])
                t2 = work.tile([128, N], wdt, tag="t2", name="t2")
                nc.vector.tensor_mul(t2, st["gfl"], st["c_e"])
                nc.vector.tensor_add(cs, t1, t2)
                t3 = work.tile([128, N], wdt, tag="t3", name="t3")
                nc.vector.tensor_mul(t3, st["gfr"], st["cob"])
                nc.vector.tensor_add(cs, cs, t3)
            tch = work.tile([128, N], wdt, tag="tch", name="tch")
            nc.scalar.activation(tch, cs, AF.Tanh)
            nc.vector.tensor_mul(h_t[lvl][:, c0 : c0 + N], st["go"], tch)

        pending = []
        for lvl, j in SEQ:
            for ch in _children(lvl, j):
                if ch in pending:
                    phase_chain(*ch)
                    pending.remove(ch)
            phase_masks(lvl, j)
            phase_body(lvl, j)
            pending.append((lvl, j))
            while len(pending) > 1:
                phase_chain(*pending.pop(0))
        for ch in pending:
            phase_chain(*ch)

        nc.sync.dma_start(out=h_out_d[:, :], in_=h_t[0][:, :BL])
        nc.sync.dma_start(out=c_out_d[:, :], in_=c_t[0][:, :BL])

    nc.finalize()
    _CACHE["nc"] = nc
    return nc


def prep_core_inputs(tokens_c, arity_c, shared):
    """Per-core input map: gather rows of precomputed vocab tables, pack masks."""
    tokens_c = np.asarray(tokens_c)
    arity_c = np.asarray(arity_c, np.int32)
    emb_bf = shared["_emb_bf"]
    hleaf_tab = shared["_hleaf_tab"]

    leaf_toks = tokens_c[:, 2**D - 1 :].reshape(-1)
    hleaf = np.ascontiguousarray(hleaf_tab[leaf_toks].T)

    xcols = []
    mcols = []
    for lvl in INT_LEVELS:
        off, cnt = 2**lvl - 1, 2**lvl
        toks = tokens_c[:, off : off + cnt].reshape(-1)
        xcols.append(emb_bf[toks].T)
        mcols.append((arity_c[:, off : off + cnt].reshape(-1) == 1).astype(BF16))
    xall = np.ascontiguousarray(np.concatenate(xcols, axis=1))
    maskb = np.concatenate(mcols)[None, :]  # [1, 8160]
    mk2 = np.concatenate([np.ones_like(maskb), maskb], axis=0)  # [2, 8160]

    out = {k: v for k, v in shared.items() if not k.startswith("_")}
    out.update(
        hleaf=hleaf,
        xall=xall,
        mk2=mk2,
        mbcast=np.broadcast_to(maskb, (128, XCOLS)).copy(),
    )
    return out


def prep_shared_inputs(emb, W, bW, Ubin, bUbin, Uun, bUun):
    emb = np.asarray(emb, np.float32)
    W = np.asarray(W, np.float32)
    bW = np.asarray(bW, np.float32)
    Ubin = np.asarray(Ubin, np.float32)
    bUbin = np.asarray(bUbin, np.float32)
    Uun = np.asarray(Uun, np.float32)
    bUun = np.asarray(bUun, np.float32)

    # per-gate [common bias b_g, arity delta d_g]; fr has no delta (mask kill)
    b_rows = np.stack(
        [
            bW[0] + bUun[0],      # i
            bW[1] + bUun[1],      # fl
            bW[1] + bUbin[2],     # fr (binary value; unary killed via cob)
            bW[2] + bUun[2],      # o
            bW[3] + bUun[3],      # u
        ]
    )
    d_rows = np.stack(
        [
            bUbin[0] - bUun[0],
            bUbin[1] - bUun[1],
            np.zeros(H, np.float32),
            bUbin[3] - bUun[2],
            bUbin[4] - bUun[3],
        ]
    )
    bd = np.stack([b_rows, d_rows]).astype(BF16)  # [2, 5, H]

    emb_bf = emb.astype(BF16)
    hleaf_tab = np.tanh(emb @ W[3] + bW[3]).astype(BF16)

    return dict(
        _emb_bf=emb_bf,
        _hleaf_tab=hleaf_tab,
        w_bf=W.astype(BF16),
        ubt_bf=Ubin[:, :H, :].astype(BF16),
        ubb_bf=Ubin[:, H:, :].astype(BF16),
        uun_bf=Uun.astype(BF16),
        bd_bf=bd,
    )


def kernel(tokens, arity, emb, W, bW, Ubin, bUbin, Uun, bUun):
    from concourse.bass_utils import run_bass_kernel_spmd

    tokens = np.asarray(tokens)
    arity = np.asarray(arity)

    shared = prep_shared_inputs(emb, W, bW, Ubin, bUbin, Uun, bUun)
    in_maps = [
        prep_core_inputs(
            tokens[k * BL : (k + 1) * BL], arity[k * BL : (k + 1) * BL], shared
        )
        for k in range(NCORES)
    ]

    nc = _build_nc()
    res = run_bass_kernel_spmd(nc, in_maps, core_ids=list(range(NCORES)))
    results = res.results

    h = np.concatenate([r["h_out"].T for r in results], axis=0)
    c = np.concatenate([r["c_out"].T for r in results], axis=0)
    return h.astype(np.float32), c.astype(np.float32)
